# revision 16
# baseline (speedup 1.0000x reference)
"""Trainium2 Bass kernel for the dual-stream video transformer block.

Sharding: 8 cores = 4 video batches x 2 halves of the patch (n) axis; each
core owns 98 of 196 patch positions for all 8 frames.  Self-attention
recomputes K/V for the full sequence (order-invariant), s2t keys come from
the input s_x, and t2s groups (b,n) are fully local -> no collectives.

Device layout is feature-major ([768 = 6x128, tokens]); every matmul
contracts over SBUF partitions, zero transposes.  Token orders:
  t stream: col = g*112 + t*14 + j   (g<7 groups of 14 owned n's)
  s stream: col = t*98 + g*14 + j, then 8 CLS at 784..792, then the other
            half's patches at 792+t*98+... (time-major).
Matmul operands are bf16 (full PE rate at any tile size); LN statistics,
softmax denominators, residual accumulation all stay fp32.  Residual
intermediates live in HBM scratch and are streamed through SBUF stripes.
K-biases are dropped (softmax-invariant); V-biases are folded into the next
projection bias on the host; softmax runs without max-subtraction (scores
are O(1) here).
"""
import sys
import numpy as np

for p in ('/opt/trn_rl_repo', '/root/.axon_site/_ro/trn_rl_repo'):
    if p not in sys.path:
        sys.path.insert(0, p)

import concourse.bass as bass  # noqa: E402
from concourse import bacc  # noqa: E402
import concourse.mybir as mybir  # noqa: E402
from concourse.tile import TileContext  # noqa: E402

F32 = mybir.dt.float32
F32R = mybir.dt.float32r
BF16 = mybir.dt.bfloat16
AF = mybir.ActivationFunctionType
OP = mybir.AluOpType

D, DC, T, NPG, NG = 768, 6, 8, 14, 7
OWN, FULL, SCOLS, SOWN = 784, 1568, 1576, 792
H, HD = 12, 64
HID, DOWN = 3072, 192
NKC = FULL // 112
EPS = 1e-6
LN1, LNS, LNT, LN2, LN2S = range(5)


def _tiles(total, size, off0=0):
    out, o = [], 0
    while o < total:
        n = min(size, total - o)
        out.append((off0 + o, n))
        o += n
    return out


def _build():
    nc = bacc.Bacc(None, target_bir_lowering=False)

    dp = lambda n, s, dt: nc.declare_dram_parameter(n, list(s), dt, isOutput=False)

    tT_d = dp("tT", [D, FULL], F32R)
    sT_d = dp("sT", [D, SCOLS], F32R)
    wd = {}
    for nm, shp in [("w_qkv", [D, 3 * D]), ("w_aproj", [D, D]),
                    ("w_s2tq", [D, D]), ("w_s2tkv", [D, 2 * D]),
                    ("w_s2tp", [D, D]), ("w_s2ta1", [D, DOWN]),
                    ("w_s2ta2", [DOWN, D]), ("w_t2sq", [D, D]),
                    ("w_t2skv", [D, 2 * D]), ("w_t2sp", [D, D]),
                    ("w_t2sa1", [D, DOWN]), ("w_t2sa2", [DOWN, D]),
                    ("w_mlp1", [D, HID]), ("w_mlp2", [HID, D]),
                    ("w_smlp1", [D, HID]), ("w_smlp2", [HID, D])]:
        wd[nm] = dp(nm, shp, BF16)
    lnpp_d = dp("lnpp", [128, 10 * DC], F32)
    qb_d = dp("qb", [128, 3 * DC], F32)
    pb_d = dp("pb", [128, 5 * DC], F32)
    mb_d = dp("mb", [128, 2 * 24 + 2 * DC], F32)
    ab_d = dp("ab", [96, 4], F32)
    pos_s_d = dp("pos_s", [128, DC, 196], F32)
    pos_v_d = dp("pos_v", [128, DC, 98], F32)
    pos_ct_d = dp("pos_ct", [128, DC, T], F32)
    pos_vt_d = dp("pos_vt", [128, DC, T], F32)
    mask_d = dp("mask", [112, 112], BF16)
    ones_d = dp("ones", [128, 448], F32R)

    tO_d = nc.declare_dram_parameter("t_out", [128, DC, OWN], F32R, isOutput=True)
    sO_d = nc.declare_dram_parameter("s_out", [128, DC, SOWN], F32R, isOutput=True)

    with TileContext(nc) as tc:
        with (
            tc.tile_pool(name="sb", bufs=1) as sb,
            tc.tile_pool(name="ps", bufs=1, space="PSUM") as ps,
            tc.tile_pool(name="dr", bufs=1, space="DRAM") as dr,
        ):
            _build_body(nc, tc, sb, ps, dr, tT_d, sT_d, wd, lnpp_d, qb_d,
                        pb_d, mb_d, ab_d, pos_s_d, pos_v_d, pos_ct_d,
                        pos_vt_d, mask_d, ones_d, tO_d, sO_d)

    nc.compile()
    return nc


def _build_body(nc, tc, sb, ps, dr, tT_d, sT_d, wd, lnpp_d, qb_d, pb_d, mb_d,
                ab_d, pos_s_d, pos_v_d, pos_ct_d, pos_vt_d, mask_d, ones_d,
                tO_d, sO_d):
    def sbt(shape, dtype, tag, bufs=1):
        return sb.tile(shape, dtype, tag=tag, bufs=bufs, name=tag)

    def pst(shape, tag):
        return ps.tile(shape, F32, tag=tag, bufs=2, name=tag)

    # ---- constants
    ones = sbt([128, 448], F32R, "ones")
    nc.sync.dma_start(ones[:], ones_d[:])
    ones_col = ones[:, 0:1]
    ones_row = ones[0:1, :]
    ones_cb = sbt([128, 1], BF16, "ones_cb")
    nc.vector.memset(ones_cb[:], 1.0)
    epsc = sbt([1, 1], F32, "epsc")
    nc.vector.memset(epsc[:], EPS)
    lnpp = sbt([128, 10 * DC], F32, "lnpp")
    nc.sync.dma_start(lnpp[:], lnpp_d[:])
    qb = sbt([128, 3 * DC], F32, "qb")
    nc.sync.dma_start(qb[:], qb_d[:])
    pb = sbt([128, 5 * DC], F32, "pb")
    nc.sync.dma_start(pb[:], pb_d[:])
    mb = sbt([128, 2 * 24 + 2 * DC], F32, "mb")
    nc.sync.dma_start(mb[:], mb_d[:])
    ab = sbt([96, 4], F32, "ab")
    nc.sync.dma_start(ab[:], ab_d[:])
    pos_s = sbt([128, DC, 196], F32, "pos_s")
    nc.sync.dma_start(pos_s[:], pos_s_d[:])
    pos_v = sbt([128, DC, 98], F32, "pos_v")
    nc.sync.dma_start(pos_v[:], pos_v_d[:])
    pos_ct = sbt([128, DC, T], F32, "pos_ct")
    nc.sync.dma_start(pos_ct[:], pos_ct_d[:])
    pos_vt = sbt([128, DC, T], F32, "pos_vt")
    nc.sync.dma_start(pos_vt[:], pos_vt_d[:])
    mask = sbt([112, 112], BF16, "mask")
    nc.sync.dma_start(mask[:], mask_d[:])

    # HBM scratch for residual intermediates, [p, c, col] layout
    t1d = dr.tile([128, DC, OWN], F32R, name="t1d")
    t2d = dr.tile([128, DC, OWN], F32R, name="t2d")
    s1d = dr.tile([128, DC, SOWN], F32R, name="s1d")

    tT_r = tT_d.rearrange("(c p) n -> p c n", p=128)
    sT_r = sT_d.rearrange("(c p) n -> p c n", p=128)

    def rload(src, c0, n):
        r = sb.tile([128, DC, 448], F32R, tag="res", bufs=2, name="res")
        nc.sync.dma_start(r[:, :, 0:n], src[:, :, c0:c0 + n])
        return r

    def wload(dram, kparts, o0, olen, kp=128, tag="wst"):
        w = sb.tile([kp, kparts, olen], BF16, tag=tag, bufs=2, name=tag)
        nc.sync.dma_start(
            w[:], dram.rearrange("(c p) o -> p c o", p=kp)[:, :, o0:o0 + olen])
        return w

    # ---------- feature-major LayerNorm on a streamed stripe ----------
    def ln_stripe(X, n, out, oc0, lni, pos=None):
        g_ap = lnpp[:, 2 * lni * DC:(2 * lni + 1) * DC]
        b_ap = lnpp[:, (2 * lni + 1) * DC:(2 * lni + 2) * DC]
        ps_x = pst([1, 448], "sa")
        ps_q = pst([1, 448], "sa")
        for c in range(DC):
            nc.tensor.matmul(ps_x[:, 0:n], ones_col, X[:, c, 0:n],
                             start=(c == 0), stop=(c == DC - 1))
        for c in range(DC):
            sq = sb.tile([128, 448], BF16, tag="ln_sq", bufs=2, name="ln_sq")
            nc.scalar.activation(sq[:, 0:n], X[:, c, 0:n], AF.Square)
            nc.tensor.matmul(ps_q[:, 0:n], ones_cb[:], sq[:, 0:n],
                             start=(c == 0), stop=(c == DC - 1))
        row = lambda dt=F32: sb.tile([1, 448], dt, tag="ln_row", bufs=6,
                                     name="ln_row")
        mrow, m2, vrow, srow = row(), row(), row(), row()
        nc.vector.tensor_scalar_mul(mrow[:, 0:n], ps_x[:, 0:n], 1.0 / D)
        nc.vector.tensor_tensor(out=m2[:, 0:n], in0=mrow[:, 0:n],
                                in1=mrow[:, 0:n], op=OP.mult)
        nc.vector.scalar_tensor_tensor(out=vrow[:, 0:n], in0=ps_q[:, 0:n],
                                       scalar=1.0 / D, in1=m2[:, 0:n],
                                       op0=OP.mult, op1=OP.subtract)
        nc.scalar.activation(srow[:, 0:n], vrow[:, 0:n], AF.Sqrt, bias=epsc[:])
        rrow, nmr = row(F32R), row(F32R)
        with nc.allow_low_precision(reason="f32r bits == f32"):
            nc.vector.reciprocal(rrow[:, 0:n], srow[:, 0:n])
        nc.vector.scalar_tensor_tensor(out=nmr[:, 0:n], in0=mrow[:, 0:n],
                                       scalar=-1.0, in1=rrow[:, 0:n],
                                       op0=OP.mult, op1=OP.mult)
        bc_r = pst([128, 448], "bc")
        nc.tensor.matmul(bc_r[:, 0:n], ones[0:1, 0:128], rrow[:, 0:n],
                         start=True, stop=True)
        bc_m = pst([128, 448], "bc")
        nc.tensor.matmul(bc_m[:, 0:n], ones[0:1, 0:128], nmr[:, 0:n],
                         start=True, stop=True)
        for c in range(DC):
            u = sb.tile([128, 448], F32, tag="ln_u", bufs=2, name="ln_u")
            nc.vector.scalar_tensor_tensor(
                out=u[:, 0:n], in0=X[:, c, 0:n], scalar=g_ap[:, c:c + 1],
                in1=bc_r[:, 0:n], op0=OP.mult, op1=OP.mult)
            v = sb.tile([128, 448], F32, tag="ln_v", bufs=1, name="ln_v")
            nc.vector.tensor_scalar(
                out=v[:, 0:n], in0=bc_m[:, 0:n], scalar1=g_ap[:, c:c + 1],
                scalar2=b_ap[:, c:c + 1], op0=OP.mult, op1=OP.add)
            if pos is None:
                nc.vector.tensor_tensor(out=out[:, c, oc0:oc0 + n],
                                        in0=u[:, 0:n], in1=v[:, 0:n], op=OP.add)
            else:
                w = sb.tile([128, 448], F32, tag="ln_u", bufs=2, name="ln_u")
                nc.vector.tensor_tensor(out=w[:, 0:n], in0=u[:, 0:n],
                                        in1=v[:, 0:n], op=OP.add)
                pv, shp = pos(c)
                nc.vector.tensor_tensor(
                    out=out[:, c, oc0:oc0 + n].rearrange(
                        "p (a b c2) -> p a b c2", b=shp[2], c2=shp[3]),
                    in0=w[:, 0:n].rearrange("p (a b c2) -> p a b c2",
                                            b=shp[2], c2=shp[3]),
                    in1=pv, op=OP.add)

    def ws_mm(w, X, c0, n, out_ap, oc, nkc=DC):
        for kc in range(nkc):
            nc.tensor.matmul(out_ap, w[:, kc, oc * 128:(oc + 1) * 128],
                             X[:, kc, c0:c0 + n],
                             start=(kc == 0), stop=(kc == nkc - 1))

    def interleave_v(v, p, half, np_):
        nc.vector.tensor_copy(
            v[:].rearrange("p (h x) -> p h x", h=H)[:, half * 6:half * 6 + 6, 0:64],
            p[0:np_, 0:384].rearrange("p (h d) -> p h d", d=64))

    def attn_norm(o_ps, qn, out_ap, reshape=None):
        rr = sb.tile([1, 448], F32R, tag="ln_row", bufs=6, name="ln_row")
        with nc.allow_low_precision(reason="f32r bits == f32"):
            nc.vector.reciprocal(rr[:, 0:qn], o_ps[64:65, 0:qn])
        bc = pst([64, 448], "bc")
        nc.tensor.matmul(bc[:, 0:qn], ones[0:1, 0:64], rr[:, 0:qn],
                         start=True, stop=True)
        bcs = sb.tile([64, 448], F32, tag="bcs", bufs=1, name="bcs")
        nc.vector.tensor_copy(bcs[:, 0:qn], bc[:, 0:qn])
        i0, i1 = o_ps[0:64, 0:qn], bcs[:, 0:qn]
        if reshape is not None:
            i0 = i0.rearrange("p (a b) -> p a b", b=reshape)
            i1 = i1.rearrange("p (a b) -> p a b", b=reshape)
        nc.vector.tensor_tensor(out=out_ap, in0=i0, in1=i1, op=OP.mult)

    def t_ap(X, c, t):
        return X[:, c, 0:OWN].rearrange(
            "p (g r) -> p g r", g=NG)[:, :, t * NPG:(t + 1) * NPG]

    def g_ap_s(X, c, g, off=0):
        return X[:, c, off:off + OWN].rearrange(
            "p (t r) -> p t r", t=T)[:, :, g * NPG:(g + 1) * NPG]

    # ================= stage 1: temporal self-attention ================
    K1 = sbt([128, DC, FULL], BF16, "K")
    Q1 = sbt([128, DC, OWN], BF16, "Q")
    V1 = [sb.tile([112, H * 65], BF16, tag=f"V_{i}", bufs=1, name="V")
          for i in range(NKC)]
    wq1 = wload(wd["w_qkv"], DC, 0, D)
    wkv1 = wload(wd["w_qkv"], DC, D, 2 * D, tag="wqkv")
    for (c0, n) in _tiles(OWN, 448) + _tiles(OWN, 448, off0=OWN):
        X = rload(tT_r, c0, n)
        l1 = sb.tile([128, DC, 448], BF16, tag="lstr", bufs=2, name="lstr")
        ln_stripe(X, n, l1, 0, LN1)
        for oc in range(DC):
            p = pst([128, 448], "mm")
            for kc in range(DC):
                nc.tensor.matmul(p[:, 0:n],
                                 wkv1[:, kc, oc * 128:(oc + 1) * 128],
                                 l1[:, kc, 0:n],
                                 start=(kc == 0), stop=(kc == DC - 1))
            nc.scalar.activation(K1[:, oc, c0:c0 + n], p[:, 0:n], AF.Copy)
        for i in range(n // 112):
            v = V1[c0 // 112 + i]
            for half in range(2):
                p = pst([128, 448], "mm")
                for kc in range(DC):
                    nc.tensor.matmul(
                        p[0:112, 0:384], l1[:, kc, i * 112:(i + 1) * 112],
                        wkv1[:, kc, D + half * 384:D + (half + 1) * 384],
                        start=(kc == 0), stop=(kc == DC - 1))
                interleave_v(v, p, half, 112)
            nc.vector.memset(
                v[:].rearrange("p (h x) -> p h x", h=H)[:, :, 64:65], 1.0)
        if c0 < OWN:
            for oc in range(DC):
                p = pst([128, 448], "mm")
                for kc in range(DC):
                    nc.tensor.matmul(p[:, 0:n],
                                     wq1[:, kc, oc * 128:(oc + 1) * 128],
                                     l1[:, kc, 0:n],
                                     start=(kc == 0), stop=(kc == DC - 1))
                nc.vector.tensor_scalar_add(Q1[:, oc, c0:c0 + n], p[:, 0:n],
                                            qb[:, oc:oc + 1])

    O1 = sbt([128, DC, OWN], BF16, "O")
    for h in range(H):
        hc, hr = h // 2, (h % 2) * 64
        for (q0, qn) in _tiles(OWN, 392):
            o_ps = pst([65, 448], "oo")
            for kc in range(NKC):
                s_ps = pst([112, 448], "sa")
                nc.tensor.matmul(s_ps[:, 0:qn],
                                 K1[hr:hr + 64, hc, kc * 112:(kc + 1) * 112],
                                 Q1[hr:hr + 64, hc, q0:q0 + qn],
                                 start=True, stop=True)
                e = sb.tile([112, 392], BF16, tag="E", bufs=2, name="E")
                nc.scalar.activation(e[:, 0:qn], s_ps[:, 0:qn], AF.Exp)
                nc.tensor.matmul(o_ps[:, 0:qn], V1[kc][:, h * 65:(h + 1) * 65],
                                 e[:, 0:qn], start=(kc == 0), stop=(kc == NKC - 1))
            attn_norm(o_ps, qn, O1[hr:hr + 64, hc, q0:q0 + qn])

    wap = wload(wd["w_aproj"], DC, 0, D)
    for (c0, n) in _tiles(OWN, 392):
        R = rload(tT_r, c0, n)
        for oc in range(DC):
            p = pst([128, 448], "mm")
            ws_mm(wap, O1, c0, n, p[:, 0:n], oc)
            u = sb.tile([128, 448], F32, tag="ev", bufs=1, name="ev")
            nc.vector.tensor_scalar_add(u[:, 0:n], p[:, 0:n], pb[:, oc:oc + 1])
            o = sb.tile([128, 448], F32R, tag="ost", bufs=2, name="ost")
            nc.vector.tensor_tensor(out=o[:, 0:n], in0=u[:, 0:n],
                                    in1=R[:, oc, 0:n], op=OP.add)
            nc.sync.dma_start(t1d[:, oc, c0:c0 + n], o[:, 0:n])

    # ================= stage 2: s2t cross-attention ====================
    Q2 = sbt([128, DC, OWN], BF16, "Q")
    wq2 = wload(wd["w_s2tq"], DC, 0, D)
    t1_r = t1d[:]
    for (c0, n) in _tiles(OWN, 448):
        X = rload(t1_r, c0, n)
        g0, ng = c0 // 112, n // 112
        tnp = sb.tile([128, DC, 448], BF16, tag="lstr", bufs=2, name="lstr")
        ln_stripe(X, n, tnp, 0, LNT,
                  pos=lambda c, g0=g0, ng=ng: (
                      pos_v[:, c, g0 * NPG:(g0 + ng) * NPG].rearrange(
                          "p (g j) -> p g () j", j=NPG).to_broadcast(
                          (128, ng, T, NPG)), (128, ng, T, NPG)))
        for oc in range(DC):
            p = pst([128, 448], "mm")
            ws_mm(wq2, tnp, 0, n, p[:, 0:n], oc)
            nc.vector.tensor_scalar_add(Q2[:, oc, c0:c0 + n], p[:, 0:n],
                                        qb[:, DC + oc:DC + oc + 1])

    K2 = sbt([128, DC, SCOLS], BF16, "K")
    V2 = [sb.tile([98, H * 65], BF16, tag=f"V_{i}", bufs=1, name="V")
          for i in range(16)]
    sn_cls = sbt([128, DC, 8], BF16, "sn_cls")
    wkv2 = wload(wd["w_s2tkv"], DC, 0, 2 * D, tag="wqkv")
    for reg in range(2):
        for (c0, n) in _tiles(OWN, 392, off0=reg * SOWN):
            X = rload(sT_r, c0, n)
            t0 = (c0 - reg * SOWN) // 98
            snp = sb.tile([128, DC, 448], BF16, tag="lstr", bufs=2, name="lstr")
            o98 = 98 * reg
            ln_stripe(X, n, snp, 0, LNS,
                      pos=lambda c, o98=o98: (
                          pos_s[:, c, o98:o98 + 98].rearrange(
                              "p r -> p () () r").to_broadcast((128, 1, 4, 98)),
                          (128, 1, 4, 98)))
            for oc in range(DC):
                p = pst([128, 448], "mm")
                ws_mm(wkv2, snp, 0, n, p[:, 0:n], oc)
                nc.scalar.activation(K2[:, oc, c0:c0 + n], p[:, 0:n], AF.Copy)
            for i in range(4):
                v = V2[(t0 + i) * 2 + reg]
                for half in range(2):
                    p = pst([128, 448], "mm")
                    for kc in range(DC):
                        nc.tensor.matmul(
                            p[0:98, 0:384], snp[:, kc, i * 98:(i + 1) * 98],
                            wkv2[:, kc, D + half * 384:D + (half + 1) * 384],
                            start=(kc == 0), stop=(kc == DC - 1))
                    interleave_v(v, p, half, 98)
                nc.vector.memset(
                    v[:].rearrange("p (h x) -> p h x", h=H)[:, :, 64:65], 1.0)
    Xc = rload(sT_r, OWN, 8)
    ln_stripe(Xc, 8, sn_cls, 0, LNS)

    O2 = sbt([128, DC, OWN], BF16, "O")
    for h in range(H):
        hc, hr = h // 2, (h % 2) * 64
        for t in range(T):
            o_ps = pst([65, 448], "oo")
            for reg in range(2):
                s_ps = pst([112, 448], "sa")
                nc.tensor.matmul(
                    s_ps[0:98, 0:98].rearrange("k (g j) -> k g j", j=NPG),
                    K2[hr:hr + 64, hc,
                       reg * SOWN + t * 98:reg * SOWN + t * 98 + 98],
                    t_ap(Q2, hc, t)[hr:hr + 64],
                    start=True, stop=True)
                e = sb.tile([112, 392], BF16, tag="E", bufs=2, name="E")
                nc.scalar.activation(e[0:98, 0:98], s_ps[0:98, 0:98], AF.Exp)
                nc.tensor.matmul(o_ps[:, 0:98],
                                 V2[t * 2 + reg][:, h * 65:(h + 1) * 65],
                                 e[0:98, 0:98], start=(reg == 0), stop=(reg == 1))
            attn_norm(o_ps, 98, t_ap(O2, hc, t)[hr:hr + 64], reshape=NPG)

    x2 = sbt([128, DC, SOWN], BF16, "X")
    wp2 = wload(wd["w_s2tp"], DC, 0, D)
    for oc in range(DC):
        for (c0, n) in _tiles(OWN, 392):
            p = pst([128, 448], "mm")
            ws_mm(wp2, O2, c0, n, p[:, 0:n], oc)
            nc.vector.tensor_scalar_add(x2[:, oc, c0:c0 + n], p[:, 0:n],
                                        pb[:, DC + oc:DC + oc + 1])
    wa1 = wload(wd["w_s2ta1"], DC, 0, DOWN)
    wa2 = wload(wd["w_s2ta2"], 2, 0, D, kp=96)
    for (c0, n) in _tiles(OWN, 392):
        ha = sb.tile([96, 2, 396], BF16, tag="ha", bufs=1, name="ha")
        for oc in range(2):
            p = pst([128, 448], "mm")
            for kc in range(DC):
                nc.tensor.matmul(p[0:96, 0:n], wa1[:, kc, oc * 96:(oc + 1) * 96],
                                 x2[:, kc, c0:c0 + n],
                                 start=(kc == 0), stop=(kc == DC - 1))
            nc.scalar.activation(ha[:, oc, 0:n], p[0:96, 0:n], AF.Gelu,
                                 bias=ab[:, oc:oc + 1])
        R = rload(t1_r, c0, n)
        for oc in range(DC):
            p = pst([128, 448], "mm")
            for kc in range(2):
                nc.tensor.matmul(p[:, 0:n], wa2[:, kc, oc * 128:(oc + 1) * 128],
                                 ha[:, kc, 0:n], start=(kc == 0), stop=(kc == 1))
            u = sb.tile([128, 448], F32, tag="ev", bufs=1, name="ev")
            nc.vector.scalar_tensor_tensor(
                out=u[:, 0:n], in0=p[:, 0:n],
                scalar=pb[:, 3 * DC + oc:3 * DC + oc + 1],
                in1=x2[:, oc, c0:c0 + n], op0=OP.add, op1=OP.add)
            o = sb.tile([128, 448], F32R, tag="ost", bufs=2, name="ost")
            nc.vector.tensor_tensor(out=o[:, 0:n], in0=u[:, 0:n],
                                    in1=R[:, oc, 0:n], op=OP.add)
            nc.sync.dma_start(t2d[:, oc, c0:c0 + n], o[:, 0:n])

    # ================= stage 3: t2s cross-attention ====================
    K3 = sbt([128, DC, OWN], BF16, "K")
    V3 = [sb.tile([112, H * 65], BF16, tag=f"V_{g}", bufs=1, name="V")
          for g in range(NG)]
    wkv3 = wload(wd["w_t2skv"], DC, 0, 2 * D, tag="wqkv")
    t2_r = t2d[:]
    for (c0, n) in _tiles(OWN, 448):
        X = rload(t2_r, c0, n)
        ng = n // 112
        tp = sb.tile([128, DC, 448], BF16, tag="lstr", bufs=2, name="lstr")
        ln_stripe(X, n, tp, 0, LNT,
                  pos=lambda c, ng=ng: (
                      pos_vt[:, c, :].rearrange("p t -> p () t ()").to_broadcast(
                          (128, ng, T, NPG)), (128, ng, T, NPG)))
        for oc in range(DC):
            p = pst([128, 448], "mm")
            ws_mm(wkv3, tp, 0, n, p[:, 0:n], oc)
            nc.scalar.activation(K3[:, oc, c0:c0 + n], p[:, 0:n], AF.Copy)
        for i in range(ng):
            v = V3[c0 // 112 + i]
            for half in range(2):
                p = pst([128, 448], "mm")
                for kc in range(DC):
                    nc.tensor.matmul(
                        p[0:112, 0:384], tp[:, kc, i * 112:(i + 1) * 112],
                        wkv3[:, kc, D + half * 384:D + (half + 1) * 384],
                        start=(kc == 0), stop=(kc == DC - 1))
                interleave_v(v, p, half, 112)
            nc.vector.memset(
                v[:].rearrange("p (h x) -> p h x", h=H)[:, :, 64:65], 1.0)

    Q3 = sbt([128, DC, OWN], BF16, "Q")
    wq3 = wload(wd["w_t2sq"], DC, 0, D)
    for (c0, n) in _tiles(OWN, 392):
        X = rload(sT_r, c0, n)
        t0 = c0 // 98
        snt = sb.tile([128, DC, 448], BF16, tag="lstr", bufs=2, name="lstr")
        ln_stripe(X, n, snt, 0, LNS,
                  pos=lambda c, t0=t0: (
                      pos_ct[:, c, t0:t0 + 4].rearrange(
                          "p t -> p () t ()").to_broadcast((128, 1, 4, 98)),
                      (128, 1, 4, 98)))
        for oc in range(DC):
            p = pst([128, 448], "mm")
            ws_mm(wq3, snt, 0, n, p[:, 0:n], oc)
            nc.vector.tensor_scalar_add(Q3[:, oc, c0:c0 + n], p[:, 0:n],
                                        qb[:, 2 * DC + oc:2 * DC + oc + 1])

    O3 = sbt([128, DC, OWN], BF16, "O")
    for h in range(H):
        hc, hr = h // 2, (h % 2) * 64
        for g in range(NG):
            s_ps = pst([112, 448], "sa")
            nc.tensor.matmul(
                s_ps[:, 0:112].rearrange("k (a b) -> k a b", b=NPG),
                K3[hr:hr + 64, hc, g * 112:(g + 1) * 112],
                g_ap_s(Q3, hc, g)[hr:hr + 64],
                start=True, stop=True)
            e = sb.tile([112, 392], BF16, tag="E", bufs=2, name="E")
            nc.scalar.activation(e[:, 0:112], s_ps[:, 0:112], AF.Exp)
            a = sb.tile([112, 112], BF16, tag="A3", bufs=1, name="A3")
            nc.vector.tensor_tensor(out=a[:], in0=e[:, 0:112], in1=mask[:],
                                    op=OP.mult)
            o_ps = pst([65, 448], "oo")
            nc.tensor.matmul(o_ps[:, 0:112], V3[g][:, h * 65:(h + 1) * 65], a[:],
                             start=True, stop=True)
            attn_norm(o_ps, 112, g_ap_s(O3, hc, g)[hr:hr + 64], reshape=NPG)

    x3 = sbt([128, DC, SOWN], BF16, "X")
    wp3 = wload(wd["w_t2sp"], DC, 0, D)
    for oc in range(DC):
        for (c0, n) in _tiles(OWN, 392):
            p = pst([128, 448], "mm")
            ws_mm(wp3, O3, c0, n, p[:, 0:n], oc)
            nc.vector.tensor_scalar_add(x3[:, oc, c0:c0 + n], p[:, 0:n],
                                        pb[:, 2 * DC + oc:2 * DC + oc + 1])
        nc.vector.tensor_copy(x3[:, oc, OWN:SOWN], sn_cls[:, oc, :])
    wa13 = wload(wd["w_t2sa1"], DC, 0, DOWN)
    wa23 = wload(wd["w_t2sa2"], 2, 0, D, kp=96)
    for (c0, n) in _tiles(SOWN, 396):
        ha = sb.tile([96, 2, 396], BF16, tag="ha", bufs=1, name="ha")
        for oc in range(2):
            p = pst([128, 448], "mm")
            for kc in range(DC):
                nc.tensor.matmul(p[0:96, 0:n], wa13[:, kc, oc * 96:(oc + 1) * 96],
                                 x3[:, kc, c0:c0 + n],
                                 start=(kc == 0), stop=(kc == DC - 1))
            nc.scalar.activation(ha[:, oc, 0:n], p[0:96, 0:n], AF.Gelu,
                                 bias=ab[:, 2 + oc:3 + oc])
        R = rload(sT_r, c0, n)
        for oc in range(DC):
            p = pst([128, 448], "mm")
            for kc in range(2):
                nc.tensor.matmul(p[:, 0:n], wa23[:, kc, oc * 128:(oc + 1) * 128],
                                 ha[:, kc, 0:n], start=(kc == 0), stop=(kc == 1))
            u = sb.tile([128, 448], F32, tag="ev", bufs=1, name="ev")
            nc.vector.scalar_tensor_tensor(
                out=u[:, 0:n], in0=p[:, 0:n],
                scalar=pb[:, 4 * DC + oc:4 * DC + oc + 1],
                in1=x3[:, oc, c0:c0 + n], op0=OP.add, op1=OP.add)
            o = sb.tile([128, 448], F32R, tag="ost", bufs=2, name="ost")
            nc.vector.tensor_tensor(out=o[:, 0:n], in0=u[:, 0:n],
                                    in1=R[:, oc, 0:n], op=OP.add)
            nc.sync.dma_start(s1d[:, oc, c0:c0 + n], o[:, 0:n])

    # ================= stage 4: MLPs ===================================
    def mlp(src_r, cols, w1_d, w2_d, mb1c, mb2c, lni, out_d):
        tsz = 392 if cols == OWN else 396
        l2 = sb.tile([128, DC, 792], BF16, tag="Q", bufs=1, name="l2")
        for (c0, n) in _tiles(cols, 448):
            X = rload(src_r, c0, n)
            ln_stripe(X, n, l2, c0, lni)
        for (c0, n) in _tiles(cols, tsz):
            hb = sb.tile([128, 24, 396], BF16, tag="K", bufs=1, name="hb")
            for sl in range(2):
                w1 = wload(w1_d, DC, sl * 1536, 1536, tag="wqkv")
                for hoc in range(12):
                    p = pst([128, 448], "mm")
                    ws_mm(w1, l2, c0, n, p[:, 0:n], hoc)
                    bi = mb1c + sl * 12 + hoc
                    nc.scalar.activation(hb[:, sl * 12 + hoc, 0:n],
                                         p[:, 0:n], AF.Gelu,
                                         bias=mb[:, bi:bi + 1])
            R = rload(src_r, c0, n)
            for oc in range(DC):
                w2 = wload(w2_d, 24, oc * 128, 128, tag="w2s")
                p = pst([128, 448], "mm")
                for kc in range(24):
                    nc.tensor.matmul(p[:, 0:n], w2[:, kc, :],
                                     hb[:, kc, 0:n],
                                     start=(kc == 0), stop=(kc == 23))
                u = sb.tile([128, 448], F32, tag="ev", bufs=1, name="ev")
                nc.vector.tensor_scalar_add(u[:, 0:n], p[:, 0:n],
                                            mb[:, mb2c + oc:mb2c + oc + 1])
                o = sb.tile([128, 448], F32R, tag="ost", bufs=2, name="ost")
                nc.vector.tensor_tensor(out=o[:, 0:n], in0=u[:, 0:n],
                                        in1=R[:, oc, 0:n], op=OP.add)
                nc.sync.dma_start(out_d[:, oc, c0:c0 + n], o[:, 0:n])

    mlp(t2_r, OWN, wd["w_mlp1"], wd["w_mlp2"], 0, 48, LN2, tO_d)
    mlp(s1d[:], SOWN, wd["w_smlp1"], wd["w_smlp2"], 24, 48 + DC, LN2S, sO_d)


# ============================ host side ================================
_B = 4
_SCALE = HD ** -0.5


def _orders(h):
    n_own = np.arange(98 * h, 98 * h + 98)
    n_oth = np.arange(98 * (1 - h), 98 * (1 - h) + 98)
    t_own = np.empty(OWN, np.int64)
    t_oth = np.empty(OWN, np.int64)
    c = 0
    for g in range(NG):
        for t in range(T):
            for j in range(NPG):
                t_own[c] = t * 196 + n_own[g * NPG + j]
                t_oth[c] = t * 196 + n_oth[g * NPG + j]
                c += 1
    s_own = np.empty(OWN, np.int64)
    s_oth = np.empty(OWN, np.int64)
    c = 0
    for t in range(T):
        for r in range(98):
            s_own[c] = t * 197 + 1 + n_own[r]
            s_oth[c] = t * 197 + 1 + n_oth[r]
            c += 1
    cls_rows = np.arange(T) * 197
    return n_own, n_oth, t_own, t_oth, s_own, s_oth, cls_rows


_nc_cache = None
LAST_EXEC_NS = None


def kernel(**inputs):
    global _nc_cache
    import ml_dtypes
    if _nc_cache is None:
        _nc_cache = _build()
    nc = _nc_cache

    f = lambda k: np.asarray(inputs[k], np.float32)
    s_x, t_x = f('s_x'), f('t_x')
    bf = lambda a: np.ascontiguousarray(a).astype(ml_dtypes.bfloat16)

    qkv_w = f('qkv_w').copy()
    qkv_w[0:D] *= _SCALE
    shared = {
        'w_qkv': bf(qkv_w.T), 'w_aproj': bf(f('attn_proj_w').T),
        'w_s2tq': bf(f('s2t_q_w').T * _SCALE), 'w_s2tkv': bf(f('s2t_kv_w').T),
        'w_s2tp': bf(f('s2t_proj_w').T), 'w_s2ta1': bf(f('s2t_a_fc1_w').T),
        'w_s2ta2': bf(f('s2t_a_fc2_w').T), 'w_t2sq': bf(f('t2s_q_w').T * _SCALE),
        'w_t2skv': bf(f('t2s_kv_w').T), 'w_t2sp': bf(f('t2s_proj_w').T),
        'w_t2sa1': bf(f('t2s_a_fc1_w').T), 'w_t2sa2': bf(f('t2s_a_fc2_w').T),
        'w_mlp1': bf(f('mlp_fc1_w').T), 'w_mlp2': bf(f('mlp_fc2_w').T),
        'w_smlp1': bf(f('smlp_fc1_w').T), 'w_smlp2': bf(f('smlp_fc2_w').T),
    }
    pp = lambda v, p=128: np.ascontiguousarray(
        np.asarray(v, np.float32).reshape(-1, p).T)
    lnpp = np.concatenate([
        np.concatenate([pp(f(g)), pp(f(b))], axis=1)
        for g, b in [('ln1_g', 'ln1_b'), ('lns_g', 'lns_b'), ('lnt_g', 'lnt_b'),
                     ('ln2_g', 'ln2_b'), ('ln2s_g', 'ln2s_b')]], axis=1)
    qbm = np.concatenate([pp(f('q_bias') * _SCALE), pp(f('s2t_q_b') * _SCALE),
                          pp(f('t2s_q_b') * _SCALE)], axis=1)
    pb_attn = f('attn_proj_b') + f('attn_proj_w') @ f('v_bias')
    pb_s2t = f('s2t_proj_b') + f('s2t_proj_w') @ f('s2t_kv_b')[D:]
    pb_t2s = f('t2s_proj_b') + f('t2s_proj_w') @ f('t2s_kv_b')[D:]
    pbm = np.concatenate([pp(pb_attn), pp(pb_s2t), pp(pb_t2s),
                          pp(f('s2t_a_fc2_b')), pp(f('t2s_a_fc2_b'))], axis=1)
    mbm = np.concatenate([pp(f('mlp_fc1_b')), pp(f('smlp_fc1_b')),
                          pp(f('mlp_fc2_b')), pp(f('smlp_fc2_b'))], axis=1)
    abm = np.concatenate([pp(f('s2t_a_fc1_b'), 96), pp(f('t2s_a_fc1_b'), 96)],
                         axis=1)
    maskm = np.zeros((112, 112), np.float32)
    for a in range(T):
        for b in range(T):
            maskm[np.ix_(a * NPG + np.arange(NPG), b * NPG + np.arange(NPG))] \
                = np.eye(NPG, dtype=np.float32)
    shared.update({'lnpp': lnpp, 'qb': qbm, 'pb': pbm, 'mb': mbm, 'ab': abm,
                   'mask': maskm.astype(ml_dtypes.bfloat16),
                   'ones': np.ones((128, 448), np.float32)})

    def pos6(v, idx=None):
        a = np.asarray(v, np.float32)
        if idx is not None:
            a = a[idx]
        return np.ascontiguousarray(a.T.reshape(DC, 128, -1).transpose(1, 0, 2))

    in_maps, meta = [], []
    for core in range(8):
        b, h = core // 2, core % 2
        n_own, n_oth, t_own, t_oth, s_own, s_oth, cls_rows = _orders(h)
        tTm = np.ascontiguousarray(t_x[b][np.concatenate([t_own, t_oth])].T)
        sb_ = s_x[b * T:(b + 1) * T].reshape(T * 197, D)
        sTm = np.ascontiguousarray(
            sb_[np.concatenate([s_own, cls_rows, s_oth])].T)
        nall = np.concatenate([n_own, n_oth])
        m = dict(shared)
        m.update({
            'tT': tTm, 'sT': sTm,
            'pos_s': pos6(f('clip_space_pos'), nall),
            'pos_v': pos6(f('vmae_space_pos'), n_own),
            'pos_ct': pos6(f('clip_time_pos')),
            'pos_vt': pos6(f('vmae_time_pos')),
        })
        in_maps.append(m)
        meta.append((b, h, t_own, s_own, cls_rows))

    from concourse.bass_utils import run_bass_kernel_spmd
    import os
    trace = bool(os.environ.get("KERNEL_TRACE"))
    res = run_bass_kernel_spmd(nc, in_maps, list(range(8)), trace=trace)
    global LAST_EXEC_NS
    LAST_EXEC_NS = getattr(res, 'exec_time_ns', None)

    s_out = np.zeros((32, 197, D), np.float32)
    t_out = np.zeros((_B, FULL, D), np.float32)
    for core in range(8):
        b, h, t_own, s_own, cls_rows = meta[core]
        r = res.results[core]
        to = np.asarray(r['t_out']).reshape(128, DC, OWN)
        to = to.transpose(1, 0, 2).reshape(D, OWN)
        so = np.asarray(r['s_out']).reshape(128, DC, SOWN)
        so = so.transpose(1, 0, 2).reshape(D, SOWN)
        t_out[b, t_own, :] = to.T
        sflat = s_out[b * T:(b + 1) * T].reshape(T * 197, D)
        sflat[s_own, :] = so.T[0:OWN]
        if h == 0:
            sflat[cls_rows, :] = so.T[OWN:SOWN]
    return s_out, t_out
